# revision 6
# baseline (speedup 1.0000x reference)
"""Distributed multi-head attention (B=2, L=2048, D=4096, H=32) on 8 TRN2 NeuronCores.

Strategy: tensor-parallel over heads (4 heads/core) for QKV+attention, then an
AllToAll that trades head-dims for token-slices so o_proj is token-sharded
(each core computes out[:, its 512 tokens] with the full Wo).

All matmuls run in bf16 on the TensorEngine (f32 PSUM accumulation).
The chip runs under a board GPIO throttle at 13/16 clock, so the PE floor for
the 43 G MACs/core is ~1.34 ms; the kernel aims to keep the PE at >98% of that.

Schedule (vs the 1.47 ms 2-phase baseline):
- Startup: group-0 x/Wq loads striped over 3 DMA queues (sync/vector/scalar),
  Wk/Wv on the gpsimd queue, cos/sin resident f16 — first MM at ~2 us.
- b=0's V never spills to DRAM: written straight into a resident SBUF tile,
  so attention b=0 starts without the 9.5 us vb reload gap.
- o_proj is split: P0 = full-contraction for b0's 256 tokens (runs during
  b1's attention, fills the exp-ACT stalls), P1..P4 = per-head passes for
  b1's tokens with SBUF f32 accumulation. Only the h=3 pass (~34 us) depends
  on the final AllToAll, and the h=2 pass hides that collective.
"""

import sys

if "/opt/trn_rl_repo" not in sys.path:
    sys.path.insert(0, "/opt/trn_rl_repo")

from contextlib import ExitStack

import ml_dtypes
import numpy as np

import concourse.bass as bass
import concourse.tile as tile
from concourse import bacc, mybir
from concourse import bass_utils

BF16 = mybir.dt.bfloat16
F16 = mybir.dt.float16
F32 = mybir.dt.float32
NPBF16 = ml_dtypes.bfloat16

NCORES = 8
B, L, D, H, HD = 2, 2048, 4096, 32, 128
T = B * L              # 4096 global tokens
NH = H // NCORES       # 4 heads per core
OC = NH * HD           # 512 projection dims per core
KT = D // 128          # 32 contraction tiles over D
LT = L // 128          # 16 key tiles per batch
TG = 512               # phase-1 token-group width
NG = T // TG           # 8 groups
SH = T // NCORES       # 512 output tokens per core
SHB = SH // B          # 256 per batch
SCALE = 1.0 / float(np.sqrt(HD))

EXP_F = mybir.ActivationFunctionType.Exp


def build_nc():
    nc = bacc.Bacc("TRN2", target_bir_lowering=False, debug=False,
                   num_devices=NCORES)

    # ---- I/O (per-core shards, host-pretiled, bf16) ----
    xT = nc.dram_tensor("xT", [KT, 128, T], BF16, kind="ExternalInput")
    wq = nc.dram_tensor("wq", [KT, 128, NH * 128], BF16, kind="ExternalInput")
    wk = nc.dram_tensor("wk", [KT, 128, NH * 128], BF16, kind="ExternalInput")
    wv = nc.dram_tensor("wv", [128, KT, OC], BF16, kind="ExternalInput")
    wo = nc.dram_tensor("wo", [D // 128, 128, KT, 128], BF16, kind="ExternalInput")
    cs = nc.dram_tensor("cs", [128, L], F16, kind="ExternalInput")
    sn = nc.dram_tensor("sn", [128, L], F16, kind="ExternalInput")
    ones = nc.dram_tensor("ones", [128, 128], BF16, kind="ExternalInput")
    out = nc.dram_tensor("out", [D, SH], F32, kind="ExternalOutput")

    # ---- internal DRAM (spills + collective bounce) ----
    qsp = [nc.dram_tensor(f"qsp{b}", [NH, 128, L], BF16) for b in range(B)]
    ksp = [nc.dram_tensor(f"ksp{b}", [NH, 128, L], BF16) for b in range(B)]
    vsp1 = nc.dram_tensor("vsp1", [LT, 128, OC], BF16)     # b=1 V spill only
    # AllToAll split by (batch, head): [shard, HD dims, SHB tokens]
    a2a_in = [[nc.dram_tensor(f"a2ai{b}_{h}", [NCORES, HD, SHB], BF16)
               for h in range(NH)] for b in range(B)]
    a2a_out = [[nc.dram_tensor(f"a2ao{b}_{h}", [NCORES, HD, SHB], BF16)
                for h in range(NH)] for b in range(B)]

    with tile.TileContext(nc) as tc, ExitStack() as ctx:
        singles = ctx.enter_context(tc.tile_pool(name="singles", bufs=1))
        ones_sb = singles.tile([128, 128], BF16, name="ones")
        nc.sync.dma_start(ones_sb[:], ones[:, :])
        cs_sb = singles.tile([128, L], F16, name="cs")
        nc.sync.dma_start(cs_sb[:], cs[:, :])
        sn_sb = singles.tile([128, L], F16, name="sn")
        nc.sync.dma_start(sn_sb[:], sn[:, :])
        # b=0's V stays resident in SBUF for the whole kernel (no DRAM spill)
        vb0 = singles.tile([128, LT, OC], BF16, name="vb0")

        # ================= Phase 1: QKV projections + RoPE =================
        with ExitStack() as p1:
            wpool = p1.enter_context(tc.tile_pool(name="w", bufs=1))
            # wk/wv stream on the gpsimd queue: needed only ~35us/~70us in
            wk_sb = wpool.tile([128, KT, NH * 128], BF16, name="wk")
            for c in range(4):
                k0 = c * (KT // 4)
                nc.gpsimd.dma_start(
                    wk_sb[:, k0:k0 + KT // 4, :],
                    wk[k0:k0 + KT // 4, :, :].transpose([1, 0, 2]))
            wv_sb = wpool.tile([128, KT, OC], BF16, name="wv")
            for c in range(4):
                k0 = c * (KT // 4)
                nc.gpsimd.dma_start(wv_sb[:, k0:k0 + KT // 4, :],
                                    wv[:, k0:k0 + KT // 4, :])
            wq_sb = wpool.tile([128, KT, NH * 128], BF16, name="wq")

            xpool = p1.enter_context(tc.tile_pool(name="xg", bufs=2))
            tmp = p1.enter_context(tc.tile_pool(name="tmp", bufs=4))
            st = p1.enter_context(tc.tile_pool(name="st", bufs=6))
            ps1 = p1.enter_context(tc.tile_pool(name="ps1", bufs=6, space="PSUM"))

            for g in range(NG):
                b = g // (NG // B)
                pos0 = (g % (NG // B)) * TG          # position within batch
                xg = xpool.tile([128, KT, TG], BF16, name="xg")
                if g == 0:
                    # stripe the critical first-group loads over all 3
                    # DMA-capable queues so the PE is fed at ~3x one queue
                    qs = (nc.sync, nc.scalar, nc.gpsimd)
                    for kt in range(KT):
                        qs[kt % 3].dma_start(xg[:, kt, :], xT[kt, :, 0:TG])
                        qs[(kt + 1) % 3].dma_start(wq_sb[:, kt, :], wq[kt, :, :])
                else:
                    nc.sync.dma_start(
                        xg[:, :, :],
                        xT[:, :, g * TG:(g + 1) * TG].transpose([1, 0, 2]))
                csg = cs_sb[:, pos0:pos0 + TG]
                sng = sn_sb[:, pos0:pos0 + TG]

                # Q and K with fused RoPE
                for wsb, sp in ((wq_sb, qsp[b]), (wk_sb, ksp[b])):
                    for pr in range(NH // 2):
                        p_re = ps1.tile([128, TG], F32, name="ps1")
                        p_im = ps1.tile([128, TG], F32, name="ps1")
                        ha, hb2 = 2 * pr, 2 * pr + 1
                        for kt in range(KT):
                            nc.tensor.matmul(p_re[:],
                                             wsb[:, kt, ha * 128:ha * 128 + 128],
                                             xg[:, kt, :],
                                             start=(kt == 0), stop=(kt == KT - 1))
                        for kt in range(KT):
                            nc.tensor.matmul(p_im[:],
                                             wsb[:, kt, hb2 * 128:hb2 * 128 + 128],
                                             xg[:, kt, :],
                                             start=(kt == 0), stop=(kt == KT - 1))
                        t1 = tmp.tile([128, TG], F32, name="tmp")
                        t2 = tmp.tile([128, TG], F32, name="tmp")
                        t3 = tmp.tile([128, TG], F32, name="tmp")
                        t4 = tmp.tile([128, TG], F32, name="tmp")
                        o_re = st.tile([128, TG], BF16, name="st")
                        o_im = st.tile([128, TG], BF16, name="st")
                        nc.vector.tensor_mul(t1[:], p_re[:], csg)
                        nc.vector.tensor_mul(t2[:], p_im[:], sng)
                        nc.vector.tensor_sub(o_re[:], t1[:], t2[:])
                        nc.vector.tensor_mul(t3[:], p_re[:], sng)
                        nc.vector.tensor_mul(t4[:], p_im[:], csg)
                        nc.vector.tensor_add(o_im[:], t3[:], t4[:])
                        ha, hb = 2 * pr, 2 * pr + 1
                        nc.gpsimd.dma_start(sp[ha, 0:64, pos0:pos0 + TG],
                                            o_re[0:64, :])
                        nc.gpsimd.dma_start(sp[hb, 0:64, pos0:pos0 + TG],
                                            o_re[64:128, :])
                        nc.gpsimd.dma_start(sp[ha, 64:128, pos0:pos0 + TG],
                                            o_im[0:64, :])
                        nc.gpsimd.dma_start(sp[hb, 64:128, pos0:pos0 + TG],
                                            o_im[64:128, :])

                # V (layout [t, oc]); b=0 goes straight to the resident tile
                for sub in range(TG // 128):
                    pv = ps1.tile([128, OC], F32, name="ps1")
                    for kt in range(KT):
                        nc.tensor.matmul(pv[:], xg[:, kt, sub * 128:(sub + 1) * 128],
                                         wv_sb[:, kt, :],
                                         start=(kt == 0), stop=(kt == KT - 1))
                    tt = pos0 // 128 + sub
                    if b == 0:
                        nc.scalar.copy(vb0[:, tt, :], pv[:])
                    else:
                        vo = st.tile([128, OC], BF16, name="st")
                        nc.scalar.copy(vo[:], pv[:])
                        nc.gpsimd.dma_start(vsp1[tt, :, :], vo[:])

        # ======== Phase 2: attention + AllToAll + o_proj (interleaved) ======
        with ExitStack() as p2:
            vbp = p2.enter_context(tc.tile_pool(name="vb", bufs=1))
            qk = p2.enter_context(tc.tile_pool(name="qk", bufs=2))
            ep = p2.enter_context(tc.tile_pool(name="ep", bufs=4))
            pvc = p2.enter_context(tc.tile_pool(name="pvc", bufs=4))
            trp = p2.enter_context(tc.tile_pool(name="tr", bufs=6))
            rc = p2.enter_context(tc.tile_pool(name="rc", bufs=4))
            ao = p2.enter_context(tc.tile_pool(name="ao", bufs=3))
            rhp = p2.enter_context(tc.tile_pool(name="rh", bufs=1))
            oaccp = p2.enter_context(tc.tile_pool(name="oacc", bufs=1))
            wop0 = p2.enter_context(tc.tile_pool(name="wo0", bufs=2))
            wop = p2.enter_context(tc.tile_pool(name="wo", bufs=2))
            osb = p2.enter_context(tc.tile_pool(name="osb", bufs=4))
            ps_s = p2.enter_context(tc.tile_pool(name="ps_s", bufs=2, space="PSUM"))
            ps_pv = p2.enter_context(tc.tile_pool(name="ps_pv", bufs=2, space="PSUM"))
            ps_o = p2.enter_context(tc.tile_pool(name="ps_o", bufs=2, space="PSUM"))

            # b=1's V reloads from DRAM during b=0's attention
            vb1 = vbp.tile([128, LT, OC], BF16, name="vb1")
            nc.sync.dma_start(vb1[:, :, :], vsp1.ap().transpose([1, 0, 2]))

            # rh: attention results re-sharded by token; global kt = 4*j + h
            rh0 = rhp.tile([128, KT, SHB], BF16, name="rh0")   # b=0 tokens
            rh1 = rhp.tile([128, KT, SHB], BF16, name="rh1")   # b=1 tokens
            rh0_4 = rh0[:].rearrange("p (j f) t -> p j f t", f=4)
            rh1_4 = rh1[:].rearrange("p (j f) t -> p j f t", f=4)
            oacc = oaccp.tile([128, D // 128, SHB], F32, name="oacc")

            def attn_head(b, h, vb):
                q_sb = qk.tile([128, L], BF16, name="q")
                nc.scalar.dma_start(q_sb[:], qsp[b][h, :, :])
                k_sb = qk.tile([128, L], BF16, name="k")
                nc.scalar.dma_start(k_sb[:], ksp[b][h, :, :])
                for half in range(2):
                    q0 = half * 1024
                    pvs = [ps_pv.tile([128, 512], F32, name="ps_pv")
                           for _ in range(2)]
                    tree = []          # bf16 pairwise row-sum tree
                    for kt in range(LT):
                        s_ps = ps_s.tile([128, 1024], F32, name="ps_s")
                        nc.tensor.matmul(s_ps[:, 0:512],
                                         k_sb[:, kt * 128:(kt + 1) * 128],
                                         q_sb[:, q0:q0 + 512],
                                         start=True, stop=True)
                        nc.tensor.matmul(s_ps[:, 512:1024],
                                         k_sb[:, kt * 128:(kt + 1) * 128],
                                         q_sb[:, q0 + 512:q0 + 1024],
                                         start=True, stop=True)
                        e_t = ep.tile([128, 1024], BF16, name="ep")
                        nc.scalar.activation(e_t[:], s_ps[:], EXP_F, scale=SCALE)
                        first, last = (kt == 0), (kt == LT - 1)
                        for c in range(2):
                            nc.tensor.matmul(pvs[c][:],
                                             vb[:, kt, h * 128:(h + 1) * 128],
                                             e_t[:, c * 512:(c + 1) * 512],
                                             start=first, stop=last)
                        node = (0, e_t)
                        while tree and tree[-1][0] == node[0]:
                            prev = tree.pop()
                            nt = trp.tile([128, 1024], BF16, name="tr")
                            nc.vector.tensor_add(nt[:], prev[1][:], node[1][:])
                            node = (node[0] + 1, nt)
                        tree.append(node)
                    assert len(tree) == 1
                    root = tree[0][1]
                    # drain pv psums to SBUF so next half's MMs start now
                    pvcs = []
                    for c in range(2):
                        pc = pvc.tile([128, 512], F32, name="pvc")
                        nc.vector.tensor_copy(pc[:], pvs[c][:])
                        pvcs.append(pc)
                    # partition-reduce the row-sum tree root (pv slots free)
                    rts = [ps_pv.tile([128, 512], F32, name="ps_pv")
                           for _ in range(2)]
                    for c in range(2):
                        nc.tensor.matmul(rts[c][:], ones_sb[:],
                                         root[:, c * 512:(c + 1) * 512],
                                         start=True, stop=True)
                    for c in range(2):
                        rec = rc.tile([128, 512], F32, name="rc")
                        nc.vector.reciprocal_approx_fast(out=rec[:],
                                                         in_=rts[c][:])
                        at = ao.tile([128, 512], BF16, name="ao")
                        nc.vector.tensor_mul(at[:], pvcs[c][:], rec[:])
                        ci = half * 2 + c
                        nc.gpsimd.dma_start(
                            a2a_in[b][h][2 * ci, :, :], at[:, 0:SHB])
                        nc.gpsimd.dma_start(
                            a2a_in[b][h][2 * ci + 1, :, :], at[:, SHB:2 * SHB])
                nc.gpsimd.collective_compute(
                    "AllToAll", mybir.AluOpType.bypass,
                    replica_groups=[list(range(NCORES))],
                    ins=[a2a_in[b][h].ap().opt()],
                    outs=[a2a_out[b][h].ap().opt()],
                )
                # gather this head's token-shard rows as they land
                rh4 = rh0_4 if b == 0 else rh1_4
                nc.sync.dma_start(rh4[:, :, h, :],
                                  a2a_out[b][h].ap().transpose([1, 0, 2]))

            def oproj_b0():
                # full-contraction o_proj for b0's 256 tokens; streams out
                for ot in range(D // 128):
                    wot = wop0.tile([128, KT, 128], BF16, name="wo0")
                    nc.sync.dma_start(wot[:, 0:KT // 2, :],
                                      wo[ot, :, 0:KT // 2, :])
                    nc.scalar.dma_start(wot[:, KT // 2:KT, :],
                                        wo[ot, :, KT // 2:KT, :])
                    po = ps_o.tile([128, SHB], F32, name="ps_o")
                    for kt in range(KT):
                        nc.tensor.matmul(po[:], wot[:, kt, :], rh0[:, kt, :],
                                         start=(kt == 0), stop=(kt == KT - 1))
                    o_sb = osb.tile([128, SHB], F32, name="osb")
                    nc.vector.tensor_copy(o_sb[:], po[:])
                    nc.gpsimd.dma_start(out[ot * 128:(ot + 1) * 128, 0:SHB],
                                        o_sb[:])

            def oproj_b1_pass(h):
                # contribution of head h (kt = 4j+h) for b1's 256 tokens
                for ot in range(D // 128):
                    wot = wop.tile([128, NCORES, 128], BF16, name="wo")
                    nc.sync.dma_start(
                        wot[:],
                        wo[ot, :, :, :].rearrange(
                            "p (j f) o -> p f j o", f=4)[:, h, :, :])
                    po = ps_o.tile([128, SHB], F32, name="ps_o")
                    for j in range(NCORES):
                        nc.tensor.matmul(po[:], wot[:, j, :],
                                         rh1_4[:, j, h, :],
                                         start=(j == 0), stop=(j == NCORES - 1))
                    if h == 0:
                        nc.scalar.copy(oacc[:, ot, :], po[:])
                    elif h < NH - 1:
                        nc.vector.tensor_add(oacc[:, ot, :], po[:],
                                             oacc[:, ot, :])
                    else:
                        o_sb = osb.tile([128, SHB], F32, name="osb")
                        nc.vector.tensor_add(o_sb[:], po[:], oacc[:, ot, :])
                        nc.gpsimd.dma_start(
                            out[ot * 128:(ot + 1) * 128, SHB:2 * SHB], o_sb[:])

            for h in range(NH):
                attn_head(0, h, vb0)
            attn_head(1, 0, vb1)
            oproj_b0()
            attn_head(1, 1, vb1)
            oproj_b1_pass(0)
            attn_head(1, 2, vb1)
            oproj_b1_pass(1)
            attn_head(1, 3, vb1)
            oproj_b1_pass(2)
            oproj_b1_pass(3)

    nc.compile()
    return nc


def _qk_row_perm():
    # local row order: [h0re|h1re],[h0im|h1im],[h2re|h3re],[h2im|h3im]
    rows = []
    for pr in range(NH // 2):
        ha, hb = 2 * pr, 2 * pr + 1
        rows += [ha * HD + 2 * i for i in range(HD // 2)]
        rows += [hb * HD + 2 * i for i in range(HD // 2)]
        rows += [ha * HD + 2 * i + 1 for i in range(HD // 2)]
        rows += [hb * HD + 2 * i + 1 for i in range(HD // 2)]
    return np.array(rows)


def _prep_inputs(x, freqs_cos, freqs_sin, Wq, Wk, Wv, Wo):
    x = np.asarray(x, np.float32).reshape(T, D)
    Wq, Wk, Wv, Wo = (np.asarray(w, np.float32) for w in (Wq, Wk, Wv, Wo))
    fc = np.asarray(freqs_cos, np.float32)
    fs = np.asarray(freqs_sin, np.float32)

    # shared tensors
    xT = np.ascontiguousarray(
        x.reshape(T, KT, 128).transpose(1, 2, 0)).astype(NPBF16)        # [KT,128,T]
    woh = np.ascontiguousarray(
        Wo.reshape(D // 128, 128, KT, 128).transpose(0, 3, 2, 1)).astype(NPBF16)
    csh = np.ascontiguousarray(np.concatenate([fc.T, fc.T], 0)).astype(np.float16)
    snh = np.ascontiguousarray(np.concatenate([fs.T, fs.T], 0)).astype(np.float16)
    ones = np.ones([128, 128], NPBF16)

    perm = _qk_row_perm()
    in_maps = []
    for i in range(NCORES):
        rows = slice(OC * i, OC * (i + 1))
        wqi = Wq[rows][perm]                                             # [512, D]
        wki = Wk[rows][perm]
        wqh = np.ascontiguousarray(
            wqi.reshape(NH * 128, KT, 128).transpose(1, 2, 0)).astype(NPBF16)
        wkh = np.ascontiguousarray(
            wki.reshape(NH * 128, KT, 128).transpose(1, 2, 0)).astype(NPBF16)
        wvh = np.ascontiguousarray(
            Wv[rows].reshape(OC, KT, 128).transpose(2, 1, 0)).astype(NPBF16)
        in_maps.append({
            "xT": xT, "wq": wqh, "wk": wkh, "wv": wvh, "wo": woh,
            "cs": csh, "sn": snh, "ones": ones,
        })
    return in_maps


_NC_CACHE = None


def _get_nc():
    global _NC_CACHE
    if _NC_CACHE is None:
        _NC_CACHE = build_nc()
    return _NC_CACHE


def _run(in_maps, trace=False):
    nc = _get_nc()
    res = bass_utils.run_bass_kernel_spmd(
        nc, in_maps, core_ids=list(range(NCORES)), trace=trace)
    return res


def _assemble(results):
    out = np.empty((B, L, D), np.float32)
    for i in range(NCORES):
        o = results[i]["out"]                       # [D, SH] f32
        for b in range(B):
            out[b, SHB * i:SHB * (i + 1), :] = o[:, b * SHB:(b + 1) * SHB].T
    return out


def kernel(x, freqs_cos, freqs_sin, Wq, Wk, Wv, Wo):
    in_maps = _prep_inputs(x, freqs_cos, freqs_sin, Wq, Wk, Wv, Wo)
    res = _run(in_maps, trace=False)
    return _assemble(res.results)


# revision 9
# speedup vs baseline: 1.0651x; 1.0651x over previous
"""Distributed multi-head attention (B=2, L=2048, D=4096, H=32) on 8 TRN2 NeuronCores.

Strategy: tensor-parallel over heads (4 heads/core) for QKV+attention, then an
AllToAll that trades head-dims for token-slices so o_proj is token-sharded
(each core computes out[:, its 512 tokens] with the full Wo).

All matmuls run in bf16 on the TensorEngine (f32 PSUM accumulation).
The chip runs under a board GPIO throttle at 13/16 clock, so the PE floor for
the 43 G MACs/core is ~1.34 ms; the kernel aims to keep the PE at >98% of that.

Schedule (vs the 1.47 ms 2-phase baseline):
- Startup: group-0 x/Wq loads striped over 3 DMA queues (sync/vector/scalar),
  Wk/Wv on the gpsimd queue, cos/sin resident f16 — first MM at ~2 us.
- b=0's V never spills to DRAM: written straight into a resident SBUF tile,
  so attention b=0 starts without the 9.5 us vb reload gap.
- o_proj is split: P0 = full-contraction for b0's 256 tokens (runs during
  b1's attention, fills the exp-ACT stalls), P1..P4 = per-head passes for
  b1's tokens with SBUF f32 accumulation. Only the h=3 pass (~34 us) depends
  on the final AllToAll, and the h=2 pass hides that collective.
"""

import sys

if "/opt/trn_rl_repo" not in sys.path:
    sys.path.insert(0, "/opt/trn_rl_repo")

from contextlib import ExitStack

import ml_dtypes
import numpy as np

import concourse.bass as bass
import concourse.tile as tile
from concourse import bacc, mybir
from concourse import bass_utils

BF16 = mybir.dt.bfloat16
F16 = mybir.dt.float16
F32 = mybir.dt.float32
NPBF16 = ml_dtypes.bfloat16

NCORES = 8
B, L, D, H, HD = 2, 2048, 4096, 32, 128
T = B * L              # 4096 global tokens
NH = H // NCORES       # 4 heads per core
OC = NH * HD           # 512 projection dims per core
KT = D // 128          # 32 contraction tiles over D
LT = L // 128          # 16 key tiles per batch
TG = 512               # phase-1 token-group width
NG = T // TG           # 8 groups
SH = T // NCORES       # 512 output tokens per core
SHB = SH // B          # 256 per batch
SCALE = 1.0 / float(np.sqrt(HD))

EXP_F = mybir.ActivationFunctionType.Exp


def build_nc():
    nc = bacc.Bacc("TRN2", target_bir_lowering=False, debug=False,
                   num_devices=NCORES)

    # ---- I/O (per-core shards, host-pretiled, bf16) ----
    xT = nc.dram_tensor("xT", [KT, 128, T], BF16, kind="ExternalInput")
    wq = nc.dram_tensor("wq", [KT, 128, NH * 128], BF16, kind="ExternalInput")
    wk = nc.dram_tensor("wk", [KT, 128, NH * 128], BF16, kind="ExternalInput")
    wv = nc.dram_tensor("wv", [128, KT, OC], BF16, kind="ExternalInput")
    wo = nc.dram_tensor("wo", [D // 128, 128, KT, 128], BF16, kind="ExternalInput")
    cs = nc.dram_tensor("cs", [128, L], F16, kind="ExternalInput")
    sn = nc.dram_tensor("sn", [128, L], F16, kind="ExternalInput")
    ones = nc.dram_tensor("ones", [128, 128], BF16, kind="ExternalInput")
    out = nc.dram_tensor("out", [D, SH], F32, kind="ExternalOutput")

    # ---- internal DRAM (spills + collective bounce) ----
    qsp = [nc.dram_tensor(f"qsp{b}", [NH, 128, L], BF16) for b in range(B)]
    ksp = [nc.dram_tensor(f"ksp{b}", [NH, 128, L], BF16) for b in range(B)]
    vsp1 = nc.dram_tensor("vsp1", [LT, 128, OC], BF16)     # b=1 V spill only
    # AllToAll split by (batch, head): [shard, HD dims, SHB tokens]
    a2a_in = [[nc.dram_tensor(f"a2ai{b}_{h}", [NCORES, HD, SHB], BF16)
               for h in range(NH)] for b in range(B)]
    a2a_out = [[nc.dram_tensor(f"a2ao{b}_{h}", [NCORES, HD, SHB], BF16)
                for h in range(NH)] for b in range(B)]

    with tile.TileContext(nc) as tc, ExitStack() as ctx:
        singles = ctx.enter_context(tc.tile_pool(name="singles", bufs=1))
        ones_sb = singles.tile([128, 128], BF16, name="ones")
        nc.sync.dma_start(ones_sb[:], ones[:, :])
        cs_sb = singles.tile([128, L], F16, name="cs")
        nc.sync.dma_start(cs_sb[:], cs[:, :])
        sn_sb = singles.tile([128, L], F16, name="sn")
        nc.sync.dma_start(sn_sb[:], sn[:, :])
        # b=0's V stays resident in SBUF for the whole kernel (no DRAM spill)
        vb0 = singles.tile([128, LT, OC], BF16, name="vb0")

        # ================= Phase 1: QKV projections + RoPE =================
        with ExitStack() as p1:
            wpool = p1.enter_context(tc.tile_pool(name="w", bufs=1))
            wk_sb = wpool.tile([128, KT, NH * 128], BF16, name="wk")
            wv_sb = wpool.tile([128, KT, OC], BF16, name="wv")
            wq_sb = wpool.tile([128, KT, NH * 128], BF16, name="wq")

            xpool = p1.enter_context(tc.tile_pool(name="xg", bufs=2))
            tmp = p1.enter_context(tc.tile_pool(name="tmp", bufs=4))
            st = p1.enter_context(tc.tile_pool(name="st", bufs=6))
            ps1 = p1.enter_context(tc.tile_pool(name="ps1", bufs=6, space="PSUM"))

            # group-0 x and Wq stripe over all 3 DMA-capable queues so the
            # first Q pass is fed at ~3x one queue's bandwidth; Wk/Wv follow
            # on gpsimd (their first use is ~35us/~70us in)
            xg0 = xpool.tile([128, KT, TG], BF16, name="xg")
            qs = (nc.sync, nc.scalar, nc.gpsimd)
            for kt in range(KT):
                qs[kt % 3].dma_start(xg0[:, kt, :], xT[kt, :, 0:TG])
                qs[(kt + 1) % 3].dma_start(wq_sb[:, kt, :], wq[kt, :, :])
            for c in range(4):
                k0 = c * (KT // 4)
                nc.gpsimd.dma_start(
                    wk_sb[:, k0:k0 + KT // 4, :],
                    wk[k0:k0 + KT // 4, :, :].transpose([1, 0, 2]))
            for c in range(4):
                k0 = c * (KT // 4)
                nc.gpsimd.dma_start(wv_sb[:, k0:k0 + KT // 4, :],
                                    wv[:, k0:k0 + KT // 4, :])

            for g in range(NG):
                b = g // (NG // B)
                pos0 = (g % (NG // B)) * TG          # position within batch
                if g == 0:
                    xg = xg0
                else:
                    xg = xpool.tile([128, KT, TG], BF16, name="xg")
                    nc.sync.dma_start(
                        xg[:, :, :],
                        xT[:, :, g * TG:(g + 1) * TG].transpose([1, 0, 2]))
                csg = cs_sb[:, pos0:pos0 + TG]
                sng = sn_sb[:, pos0:pos0 + TG]

                # Q and K with fused RoPE
                for wsb, sp in ((wq_sb, qsp[b]), (wk_sb, ksp[b])):
                    for pr in range(NH // 2):
                        p_re = ps1.tile([128, TG], F32, name="ps1")
                        p_im = ps1.tile([128, TG], F32, name="ps1")
                        ha, hb2 = 2 * pr, 2 * pr + 1
                        for kt in range(KT):
                            nc.tensor.matmul(p_re[:],
                                             wsb[:, kt, ha * 128:ha * 128 + 128],
                                             xg[:, kt, :],
                                             start=(kt == 0), stop=(kt == KT - 1))
                        for kt in range(KT):
                            nc.tensor.matmul(p_im[:],
                                             wsb[:, kt, hb2 * 128:hb2 * 128 + 128],
                                             xg[:, kt, :],
                                             start=(kt == 0), stop=(kt == KT - 1))
                        t1 = tmp.tile([128, TG], F32, name="tmp")
                        t2 = tmp.tile([128, TG], F32, name="tmp")
                        t3 = tmp.tile([128, TG], F32, name="tmp")
                        t4 = tmp.tile([128, TG], F32, name="tmp")
                        o_re = st.tile([128, TG], BF16, name="st")
                        o_im = st.tile([128, TG], BF16, name="st")
                        nc.vector.tensor_mul(t1[:], p_re[:], csg)
                        nc.vector.tensor_mul(t2[:], p_im[:], sng)
                        nc.vector.tensor_sub(o_re[:], t1[:], t2[:])
                        nc.vector.tensor_mul(t3[:], p_re[:], sng)
                        nc.vector.tensor_mul(t4[:], p_im[:], csg)
                        nc.vector.tensor_add(o_im[:], t3[:], t4[:])
                        ha, hb = 2 * pr, 2 * pr + 1
                        nc.gpsimd.dma_start(sp[ha, 0:64, pos0:pos0 + TG],
                                            o_re[0:64, :])
                        nc.gpsimd.dma_start(sp[hb, 0:64, pos0:pos0 + TG],
                                            o_re[64:128, :])
                        nc.gpsimd.dma_start(sp[ha, 64:128, pos0:pos0 + TG],
                                            o_im[0:64, :])
                        nc.gpsimd.dma_start(sp[hb, 64:128, pos0:pos0 + TG],
                                            o_im[64:128, :])

                # V (layout [t, oc]); b=0 goes straight to the resident tile
                for sub in range(TG // 128):
                    pv = ps1.tile([128, OC], F32, name="ps1")
                    for kt in range(KT):
                        nc.tensor.matmul(pv[:], xg[:, kt, sub * 128:(sub + 1) * 128],
                                         wv_sb[:, kt, :],
                                         start=(kt == 0), stop=(kt == KT - 1))
                    tt = pos0 // 128 + sub
                    if b == 0:
                        nc.scalar.copy(vb0[:, tt, :], pv[:])
                    else:
                        vo = st.tile([128, OC], BF16, name="st")
                        nc.scalar.copy(vo[:], pv[:])
                        nc.gpsimd.dma_start(vsp1[tt, :, :], vo[:])

        # ======== Phase 2: attention + AllToAll + o_proj (interleaved) ======
        with ExitStack() as p2:
            vbp = p2.enter_context(tc.tile_pool(name="vb", bufs=1))
            qk = p2.enter_context(tc.tile_pool(name="qk", bufs=2))
            ep = p2.enter_context(tc.tile_pool(name="ep", bufs=4))
            pvc = p2.enter_context(tc.tile_pool(name="pvc", bufs=4))
            trp = p2.enter_context(tc.tile_pool(name="tr", bufs=6))
            rc = p2.enter_context(tc.tile_pool(name="rc", bufs=4))
            ao = p2.enter_context(tc.tile_pool(name="ao", bufs=3))
            rhp = p2.enter_context(tc.tile_pool(name="rh", bufs=2))
            oaccp = p2.enter_context(tc.tile_pool(name="oacc", bufs=1))
            wop = p2.enter_context(tc.tile_pool(name="wo", bufs=4))
            osb = p2.enter_context(tc.tile_pool(name="osb", bufs=2))
            ps_s = p2.enter_context(tc.tile_pool(name="ps_s", bufs=2, space="PSUM"))
            ps_pv = p2.enter_context(tc.tile_pool(name="ps_pv", bufs=2, space="PSUM"))
            ps_o = p2.enter_context(tc.tile_pool(name="ps_o", bufs=2, space="PSUM"))

            # per-batch o_proj accumulators (256 tokens each)
            oacc0 = oaccp.tile([128, D // 128, SHB], F32, name="oacc0")
            oacc1 = oaccp.tile([128, D // 128, SHB], F32, name="oacc1")

            def attn_head(b, h, vb):
                q_sb = qk.tile([128, L], BF16, name="q")
                nc.sync.dma_start(q_sb[:], qsp[b][h, :, :])
                k_sb = qk.tile([128, L], BF16, name="k")
                nc.sync.dma_start(k_sb[:], ksp[b][h, :, :])
                for half in range(2):
                    q0 = half * 1024
                    pvs = [ps_pv.tile([128, 512], F32, name="ps_pv")
                           for _ in range(2)]
                    tree = []          # bf16 pairwise row-sum tree
                    for kt in range(LT):
                        s_ps = ps_s.tile([128, 1024], F32, name="ps_s")
                        nc.tensor.matmul(s_ps[:, 0:512],
                                         k_sb[:, kt * 128:(kt + 1) * 128],
                                         q_sb[:, q0:q0 + 512],
                                         start=True, stop=True)
                        nc.tensor.matmul(s_ps[:, 512:1024],
                                         k_sb[:, kt * 128:(kt + 1) * 128],
                                         q_sb[:, q0 + 512:q0 + 1024],
                                         start=True, stop=True)
                        e_t = ep.tile([128, 1024], BF16, name="ep")
                        nc.scalar.activation(e_t[:], s_ps[:], EXP_F, scale=SCALE)
                        first, last = (kt == 0), (kt == LT - 1)
                        for c in range(2):
                            nc.tensor.matmul(pvs[c][:],
                                             vb[:, kt, h * 128:(h + 1) * 128],
                                             e_t[:, c * 512:(c + 1) * 512],
                                             start=first, stop=last)
                        node = (0, e_t)
                        while tree and tree[-1][0] == node[0]:
                            prev = tree.pop()
                            nt = trp.tile([128, 1024], BF16, name="tr")
                            nc.vector.tensor_add(nt[:], prev[1][:], node[1][:])
                            node = (node[0] + 1, nt)
                        tree.append(node)
                    assert len(tree) == 1
                    root = tree[0][1]
                    # drain pv psums to SBUF so next half's MMs start now
                    pvcs = []
                    for c in range(2):
                        pc = pvc.tile([128, 512], F32, name="pvc")
                        nc.vector.tensor_copy(pc[:], pvs[c][:])
                        pvcs.append(pc)
                    # partition-reduce the row-sum tree root (pv slots free)
                    rts = [ps_pv.tile([128, 512], F32, name="ps_pv")
                           for _ in range(2)]
                    for c in range(2):
                        nc.tensor.matmul(rts[c][:], ones_sb[:],
                                         root[:, c * 512:(c + 1) * 512],
                                         start=True, stop=True)
                    for c in range(2):
                        rec = rc.tile([128, 512], F32, name="rc")
                        nc.vector.reciprocal_approx_fast(out=rec[:],
                                                         in_=rts[c][:])
                        at = ao.tile([128, 512], BF16, name="ao")
                        nc.vector.tensor_mul(at[:], pvcs[c][:], rec[:])
                        ci = half * 2 + c
                        nc.gpsimd.dma_start(
                            a2a_in[b][h][2 * ci, :, :], at[:, 0:SHB])
                        nc.gpsimd.dma_start(
                            a2a_in[b][h][2 * ci + 1, :, :], at[:, SHB:2 * SHB])
                nc.gpsimd.collective_compute(
                    "AllToAll", mybir.AluOpType.bypass,
                    replica_groups=[list(range(NCORES))],
                    ins=[a2a_in[b][h].ap().opt()],
                    outs=[a2a_out[b][h].ap().opt()],
                )

            def oproj_pass(b, h):
                # contribution of local head h (global kt = 4j+h) to batch b's
                # 256 tokens.  Emitted ~2 heads after attn(b,h) so the a2a
                # (~40us) is complete; the pass's MMs then fill the exp-ACT
                # stalls of the concurrently-running attention head.
                rh_t = rhp.tile([128, NCORES, SHB], BF16, name="rh")
                nc.sync.dma_start(rh_t[:], a2a_out[b][h].ap().transpose([1, 0, 2]))
                oa = oacc0 if b == 0 else oacc1
                col0 = b * SHB
                for ot in range(D // 128):
                    wot = wop.tile([128, NCORES, 128], BF16, name="wo")
                    nc.sync.dma_start(
                        wot[:],
                        wo[ot, :, :, :].rearrange(
                            "p (j f) o -> p f j o", f=4)[:, h, :, :])
                    po = ps_o.tile([128, SHB], F32, name="ps_o")
                    for j in range(NCORES):
                        nc.tensor.matmul(po[:], wot[:, j, :], rh_t[:, j, :],
                                         start=(j == 0), stop=(j == NCORES - 1))
                    if h == 0:
                        nc.vector.tensor_copy(oa[:, ot, :], po[:])
                    elif h < NH - 1:
                        nc.vector.tensor_add(oa[:, ot, :], po[:],
                                             oa[:, ot, :])
                    else:
                        o_sb = osb.tile([128, SHB], F32, name="osb")
                        nc.vector.tensor_add(o_sb[:], po[:], oa[:, ot, :])
                        nc.gpsimd.dma_start(
                            out[ot * 128:(ot + 1) * 128, col0:col0 + SHB],
                            o_sb[:])

            attn_head(0, 0, vb0)
            attn_head(0, 1, vb0)
            # b=1's V reloads from DRAM well before b1's attention needs it
            vb1 = vbp.tile([128, LT, OC], BF16, name="vb1")
            nc.sync.dma_start(vb1[:, :, :], vsp1.ap().transpose([1, 0, 2]))
            attn_head(0, 2, vb0)
            oproj_pass(0, 0)
            attn_head(0, 3, vb0)
            oproj_pass(0, 1)
            attn_head(1, 0, vb1)
            oproj_pass(0, 2)
            attn_head(1, 1, vb1)
            oproj_pass(0, 3)
            attn_head(1, 2, vb1)
            oproj_pass(1, 0)
            attn_head(1, 3, vb1)
            oproj_pass(1, 1)
            oproj_pass(1, 2)
            oproj_pass(1, 3)

    nc.compile()
    return nc


def _qk_row_perm():
    # local row order: [h0re|h1re],[h0im|h1im],[h2re|h3re],[h2im|h3im]
    rows = []
    for pr in range(NH // 2):
        ha, hb = 2 * pr, 2 * pr + 1
        rows += [ha * HD + 2 * i for i in range(HD // 2)]
        rows += [hb * HD + 2 * i for i in range(HD // 2)]
        rows += [ha * HD + 2 * i + 1 for i in range(HD // 2)]
        rows += [hb * HD + 2 * i + 1 for i in range(HD // 2)]
    return np.array(rows)


def _prep_inputs(x, freqs_cos, freqs_sin, Wq, Wk, Wv, Wo):
    x = np.asarray(x, np.float32).reshape(T, D)
    Wq, Wk, Wv, Wo = (np.asarray(w, np.float32) for w in (Wq, Wk, Wv, Wo))
    fc = np.asarray(freqs_cos, np.float32)
    fs = np.asarray(freqs_sin, np.float32)

    # shared tensors
    xT = np.ascontiguousarray(
        x.reshape(T, KT, 128).transpose(1, 2, 0)).astype(NPBF16)        # [KT,128,T]
    woh = np.ascontiguousarray(
        Wo.reshape(D // 128, 128, KT, 128).transpose(0, 3, 2, 1)).astype(NPBF16)
    csh = np.ascontiguousarray(np.concatenate([fc.T, fc.T], 0)).astype(np.float16)
    snh = np.ascontiguousarray(np.concatenate([fs.T, fs.T], 0)).astype(np.float16)
    ones = np.ones([128, 128], NPBF16)

    perm = _qk_row_perm()
    in_maps = []
    for i in range(NCORES):
        rows = slice(OC * i, OC * (i + 1))
        wqi = Wq[rows][perm]                                             # [512, D]
        wki = Wk[rows][perm]
        wqh = np.ascontiguousarray(
            wqi.reshape(NH * 128, KT, 128).transpose(1, 2, 0)).astype(NPBF16)
        wkh = np.ascontiguousarray(
            wki.reshape(NH * 128, KT, 128).transpose(1, 2, 0)).astype(NPBF16)
        wvh = np.ascontiguousarray(
            Wv[rows].reshape(OC, KT, 128).transpose(2, 1, 0)).astype(NPBF16)
        in_maps.append({
            "xT": xT, "wq": wqh, "wk": wkh, "wv": wvh, "wo": woh,
            "cs": csh, "sn": snh, "ones": ones,
        })
    return in_maps


_NC_CACHE = None


def _get_nc():
    global _NC_CACHE
    if _NC_CACHE is None:
        _NC_CACHE = build_nc()
    return _NC_CACHE


def _run(in_maps, trace=False):
    nc = _get_nc()
    res = bass_utils.run_bass_kernel_spmd(
        nc, in_maps, core_ids=list(range(NCORES)), trace=trace)
    return res


def _assemble(results):
    out = np.empty((B, L, D), np.float32)
    for i in range(NCORES):
        o = results[i]["out"]                       # [D, SH] f32
        for b in range(B):
            out[b, SHB * i:SHB * (i + 1), :] = o[:, b * SHB:(b + 1) * SHB].T
    return out


def kernel(x, freqs_cos, freqs_sin, Wq, Wk, Wv, Wo):
    in_maps = _prep_inputs(x, freqs_cos, freqs_sin, Wq, Wk, Wv, Wo)
    res = _run(in_maps, trace=False)
    return _assemble(res.results)


# revision 12
# speedup vs baseline: 1.1318x; 1.0626x over previous
"""Distributed multi-head attention (B=2, L=2048, D=4096, H=32) on 8 TRN2 NeuronCores.

Strategy: tensor-parallel over heads (4 heads/core) for QKV+attention, then an
AllToAll that trades head-dims for token-slices so o_proj is token-sharded
(each core computes out[:, its 512 tokens] with the full Wo).

All matmuls run in bf16 on the TensorEngine (f32 PSUM accumulation).
The chip runs under a board GPIO throttle at 13/16 clock, so the PE floor for
the 43 G MACs/core is ~1.34 ms; the kernel aims to keep the PE at >98% of that.

Key scheduling points (vs the 1.47 ms 2-phase baseline):
- All large loads use host-pretiled PARTITION-MAJOR layouts (>=32KB contiguous
  per partition) so each DMA queue sustains full bandwidth; the startup x/Wq
  loads are striped in kt-ranges across all 3 DMA-capable queues.
- b=0's V never spills to DRAM (written straight into a resident SBUF tile);
  cos/sin are resident f16.
- o_proj is split into 8 passes, one per (batch, head): pass (b,h) covers
  contraction tiles kt=4j+h for batch b's 256 tokens (8 MMs of N=256 per
  output tile, full-rate).  Each pass is emitted ~3 attention-heads after
  attn(b,h) so its AllToAll (~57us completion) is done, and its MMs fill the
  exp-ACT stalls of the concurrently-running attention head.  Only the last
  pass (~34us) depends on the final AllToAll, and two earlier passes hide it.
- Queue discipline: scalar queue = q/k prefetch + exp ACTs only; sync queue =
  throughput loads (whole-pass wo tiles, rh gathers, vb1); gpsimd = spills,
  a2a stores, collectives, output stores.  Collective-gated gathers never sit
  ahead of attention-critical DMAs in any FIFO.
"""

import sys

if "/opt/trn_rl_repo" not in sys.path:
    sys.path.insert(0, "/opt/trn_rl_repo")

from contextlib import ExitStack

import ml_dtypes
import numpy as np

import concourse.bass as bass
import concourse.tile as tile
from concourse import bacc, mybir
from concourse import bass_utils

BF16 = mybir.dt.bfloat16
F16 = mybir.dt.float16
F32 = mybir.dt.float32
NPBF16 = ml_dtypes.bfloat16

NCORES = 8
B, L, D, H, HD = 2, 2048, 4096, 32, 128
T = B * L              # 4096 global tokens
NH = H // NCORES       # 4 heads per core
OC = NH * HD           # 512 projection dims per core
KT = D // 128          # 32 contraction tiles over D
LT = L // 128          # 16 key tiles per batch
TG = 512               # phase-1 token-group width
NG = T // TG           # 8 groups
SH = T // NCORES       # 512 output tokens per core
SHB = SH // B          # 256 per batch
SCALE = 1.0 / float(np.sqrt(HD))

EXP_F = mybir.ActivationFunctionType.Exp


def build_nc():
    nc = bacc.Bacc("TRN2", target_bir_lowering=False, debug=False,
                   num_devices=NCORES)

    # ---- I/O (per-core shards, host-pretiled partition-major, bf16) ----
    xH = nc.dram_tensor("xH", [NG, 128, KT, TG], BF16, kind="ExternalInput")
    wq = nc.dram_tensor("wq", [128, KT, NH * 128], BF16, kind="ExternalInput")
    wk = nc.dram_tensor("wk", [128, KT, NH * 128], BF16, kind="ExternalInput")
    wv = nc.dram_tensor("wv", [128, KT, OC], BF16, kind="ExternalInput")
    # wo[h][p][ot][j][o] = Wo[ot*128+o, (4j+h)*128+p]
    wo = nc.dram_tensor("wo", [NH, 128, D // 128, NCORES, 128], BF16,
                        kind="ExternalInput")
    cs = nc.dram_tensor("cs", [128, L], F16, kind="ExternalInput")
    sn = nc.dram_tensor("sn", [128, L], F16, kind="ExternalInput")
    ones = nc.dram_tensor("ones", [128, 128], BF16, kind="ExternalInput")
    out = nc.dram_tensor("out", [D, SH], F32, kind="ExternalOutput")

    # ---- internal DRAM (spills + collective bounce) ----
    qsp = [nc.dram_tensor(f"qsp{b}", [NH, 128, L], BF16) for b in range(B)]
    ksp = [nc.dram_tensor(f"ksp{b}", [NH, 128, L], BF16) for b in range(B)]
    vsp1 = nc.dram_tensor("vsp1", [128, LT, OC], BF16)     # b=1 V spill only
    # AllToAll split by (batch, head): [shard, HD dims, SHB tokens]
    a2a_in = [[nc.dram_tensor(f"a2ai{b}_{h}", [NCORES, HD, SHB], BF16)
               for h in range(NH)] for b in range(B)]
    a2a_out = [[nc.dram_tensor(f"a2ao{b}_{h}", [NCORES, HD, SHB], BF16)
                for h in range(NH)] for b in range(B)]

    with tile.TileContext(nc) as tc, ExitStack() as ctx:
        singles = ctx.enter_context(tc.tile_pool(name="singles", bufs=1))
        ones_sb = singles.tile([128, 128], BF16, name="ones")
        cs_sb = singles.tile([128, L], F16, name="cs")
        sn_sb = singles.tile([128, L], F16, name="sn")
        # b=0's V stays resident in SBUF for the whole kernel (no DRAM spill)
        vb0 = singles.tile([128, LT, OC], BF16, name="vb0")

        # ================= Phase 1: QKV projections + RoPE =================
        with ExitStack() as p1:
            wpool = p1.enter_context(tc.tile_pool(name="w", bufs=1))
            wk_sb = wpool.tile([128, KT, NH * 128], BF16, name="wk")
            wv_sb = wpool.tile([128, KT, OC], BF16, name="wv")
            wq_sb = wpool.tile([128, KT, NH * 128], BF16, name="wq")

            xpool = p1.enter_context(tc.tile_pool(name="xg", bufs=2))
            tmp = p1.enter_context(tc.tile_pool(name="tmp", bufs=4))
            st = p1.enter_context(tc.tile_pool(name="st", bufs=6))
            ps1 = p1.enter_context(tc.tile_pool(name="ps1", bufs=6, space="PSUM"))

            # Startup: stripe group-0 x / Wq in kt-ranges over all 3 queues,
            # ordered so each tile lands just before its first MM consumes it.
            xg0 = xpool.tile([128, KT, TG], BF16, name="xg")
            t1_, t2_ = 11, 22
            nc.sync.dma_start(xg0[:, 0:t1_, :], xH[0, :, 0:t1_, :])
            nc.sync.dma_start(wq_sb[:, t1_:t2_, :], wq[:, t1_:t2_, :])
            nc.scalar.dma_start(wq_sb[:, 0:t1_, :], wq[:, 0:t1_, :])
            nc.scalar.dma_start(xg0[:, t1_:t2_, :], xH[0, :, t1_:t2_, :])
            nc.scalar.dma_start(cs_sb[:], cs[:, :])
            nc.scalar.dma_start(sn_sb[:], sn[:, :])
            nc.gpsimd.dma_start(xg0[:, t2_:KT, :], xH[0, :, t2_:KT, :])
            nc.gpsimd.dma_start(wq_sb[:, t2_:KT, :], wq[:, t2_:KT, :])
            for c in range(2):
                k0 = c * (KT // 2)
                nc.gpsimd.dma_start(wk_sb[:, k0:k0 + KT // 2, :],
                                    wk[:, k0:k0 + KT // 2, :])
            for c in range(2):
                k0 = c * (KT // 2)
                nc.gpsimd.dma_start(wv_sb[:, k0:k0 + KT // 2, :],
                                    wv[:, k0:k0 + KT // 2, :])
            nc.gpsimd.dma_start(ones_sb[:], ones[:, :])

            for g in range(NG):
                b = g // (NG // B)
                pos0 = (g % (NG // B)) * TG          # position within batch
                if g == 0:
                    xg = xg0
                else:
                    xg = xpool.tile([128, KT, TG], BF16, name="xg")
                    nc.sync.dma_start(xg[:, :, :], xH[g, :, :, :])
                csg = cs_sb[:, pos0:pos0 + TG]
                sng = sn_sb[:, pos0:pos0 + TG]

                # Q and K with fused RoPE
                for wsb, sp in ((wq_sb, qsp[b]), (wk_sb, ksp[b])):
                    for pr in range(NH // 2):
                        p_re = ps1.tile([128, TG], F32, name="ps1")
                        p_im = ps1.tile([128, TG], F32, name="ps1")
                        ha, hb2 = 2 * pr, 2 * pr + 1
                        for kt in range(KT):
                            nc.tensor.matmul(p_re[:],
                                             wsb[:, kt, ha * 128:ha * 128 + 128],
                                             xg[:, kt, :],
                                             start=(kt == 0), stop=(kt == KT - 1))
                        for kt in range(KT):
                            nc.tensor.matmul(p_im[:],
                                             wsb[:, kt, hb2 * 128:hb2 * 128 + 128],
                                             xg[:, kt, :],
                                             start=(kt == 0), stop=(kt == KT - 1))
                        t1 = tmp.tile([128, TG], F32, name="tmp")
                        t2 = tmp.tile([128, TG], F32, name="tmp")
                        t3 = tmp.tile([128, TG], F32, name="tmp")
                        t4 = tmp.tile([128, TG], F32, name="tmp")
                        o_re = st.tile([128, TG], BF16, name="st")
                        o_im = st.tile([128, TG], BF16, name="st")
                        nc.vector.tensor_mul(t1[:], p_re[:], csg)
                        nc.vector.tensor_mul(t2[:], p_im[:], sng)
                        nc.vector.tensor_sub(o_re[:], t1[:], t2[:])
                        nc.vector.tensor_mul(t3[:], p_re[:], sng)
                        nc.vector.tensor_mul(t4[:], p_im[:], csg)
                        nc.vector.tensor_add(o_im[:], t3[:], t4[:])
                        ha, hb = 2 * pr, 2 * pr + 1
                        nc.gpsimd.dma_start(sp[ha, 0:64, pos0:pos0 + TG],
                                            o_re[0:64, :])
                        nc.gpsimd.dma_start(sp[hb, 0:64, pos0:pos0 + TG],
                                            o_re[64:128, :])
                        nc.gpsimd.dma_start(sp[ha, 64:128, pos0:pos0 + TG],
                                            o_im[0:64, :])
                        nc.gpsimd.dma_start(sp[hb, 64:128, pos0:pos0 + TG],
                                            o_im[64:128, :])

                # V (layout [t, oc]); b=0 goes straight to the resident tile
                for sub in range(TG // 128):
                    pv = ps1.tile([128, OC], F32, name="ps1")
                    for kt in range(KT):
                        nc.tensor.matmul(pv[:], xg[:, kt, sub * 128:(sub + 1) * 128],
                                         wv_sb[:, kt, :],
                                         start=(kt == 0), stop=(kt == KT - 1))
                    tt = pos0 // 128 + sub
                    if b == 0:
                        nc.scalar.copy(vb0[:, tt, :], pv[:])
                    else:
                        vo = st.tile([128, OC], BF16, name="st")
                        nc.scalar.copy(vo[:], pv[:])
                        nc.gpsimd.dma_start(vsp1[:, tt, :], vo[:])

        # ======== Phase 2: attention + AllToAll + o_proj (interleaved) ======
        with ExitStack() as p2:
            vbp = p2.enter_context(tc.tile_pool(name="vb", bufs=1))
            qk = p2.enter_context(tc.tile_pool(name="qk", bufs=2))
            ep = p2.enter_context(tc.tile_pool(name="ep", bufs=4))
            pvc = p2.enter_context(tc.tile_pool(name="pvc", bufs=4))
            trp = p2.enter_context(tc.tile_pool(name="tr", bufs=6))
            rc = p2.enter_context(tc.tile_pool(name="rc", bufs=4))
            ao = p2.enter_context(tc.tile_pool(name="ao", bufs=6))
            rhp = p2.enter_context(tc.tile_pool(name="rh", bufs=2))
            oaccp = p2.enter_context(tc.tile_pool(name="oacc", bufs=1))
            wop = p2.enter_context(tc.tile_pool(name="wo", bufs=2))
            osb = p2.enter_context(tc.tile_pool(name="osb", bufs=2))
            ps_s = p2.enter_context(tc.tile_pool(name="ps_s", bufs=2, space="PSUM"))
            ps_pv = p2.enter_context(tc.tile_pool(name="ps_pv", bufs=2, space="PSUM"))
            ps_o = p2.enter_context(tc.tile_pool(name="ps_o", bufs=2, space="PSUM"))

            # per-batch o_proj accumulators (256 tokens each)
            oacc0 = oaccp.tile([128, D // 128, SHB], F32, name="oacc0")
            oacc1 = oaccp.tile([128, D // 128, SHB], F32, name="oacc1")

            def attn_head(b, h, vb):
                q_sb = qk.tile([128, L], BF16, name="q")
                nc.scalar.dma_start(q_sb[:], qsp[b][h, :, :])
                k_sb = qk.tile([128, L], BF16, name="k")
                nc.scalar.dma_start(k_sb[:], ksp[b][h, :, :])
                for half in range(2):
                    q0 = half * 1024
                    pvs = [ps_pv.tile([128, 512], F32, name="ps_pv")
                           for _ in range(2)]
                    tree = []          # bf16 pairwise row-sum tree
                    for kt in range(LT):
                        s_ps = ps_s.tile([128, 1024], F32, name="ps_s")
                        nc.tensor.matmul(s_ps[:, 0:512],
                                         k_sb[:, kt * 128:(kt + 1) * 128],
                                         q_sb[:, q0:q0 + 512],
                                         start=True, stop=True)
                        nc.tensor.matmul(s_ps[:, 512:1024],
                                         k_sb[:, kt * 128:(kt + 1) * 128],
                                         q_sb[:, q0 + 512:q0 + 1024],
                                         start=True, stop=True)
                        e_t = ep.tile([128, 1024], BF16, name="ep")
                        nc.scalar.activation(e_t[:], s_ps[:], EXP_F, scale=SCALE)
                        first, last = (kt == 0), (kt == LT - 1)
                        for c in range(2):
                            nc.tensor.matmul(pvs[c][:],
                                             vb[:, kt, h * 128:(h + 1) * 128],
                                             e_t[:, c * 512:(c + 1) * 512],
                                             start=first, stop=last)
                        node = (0, e_t)
                        while tree and tree[-1][0] == node[0]:
                            prev = tree.pop()
                            nt = trp.tile([128, 1024], BF16, name="tr")
                            nc.vector.tensor_add(nt[:], prev[1][:], node[1][:])
                            node = (node[0] + 1, nt)
                        tree.append(node)
                    assert len(tree) == 1
                    root = tree[0][1]
                    # drain pv psums to SBUF so next half's MMs start now
                    pvcs = []
                    for c in range(2):
                        pc = pvc.tile([128, 512], F32, name="pvc")
                        nc.vector.tensor_copy(pc[:], pvs[c][:])
                        pvcs.append(pc)
                    # partition-reduce the row-sum tree root (pv slots free)
                    rts = [ps_pv.tile([128, 512], F32, name="ps_pv")
                           for _ in range(2)]
                    for c in range(2):
                        nc.tensor.matmul(rts[c][:], ones_sb[:],
                                         root[:, c * 512:(c + 1) * 512],
                                         start=True, stop=True)
                    for c in range(2):
                        rec = rc.tile([128, 512], F32, name="rc")
                        nc.vector.reciprocal_approx_fast(out=rec[:],
                                                         in_=rts[c][:])
                        at = ao.tile([128, 512], BF16, name="ao")
                        nc.vector.tensor_mul(at[:], pvcs[c][:], rec[:])
                        ci = half * 2 + c
                        nc.gpsimd.dma_start(
                            a2a_in[b][h][2 * ci, :, :], at[:, 0:SHB])
                        nc.gpsimd.dma_start(
                            a2a_in[b][h][2 * ci + 1, :, :], at[:, SHB:2 * SHB])
                nc.gpsimd.collective_compute(
                    "AllToAll", mybir.AluOpType.bypass,
                    replica_groups=[list(range(NCORES))],
                    ins=[a2a_in[b][h].ap().opt()],
                    outs=[a2a_out[b][h].ap().opt()],
                )

            def oproj_pass(b, h):
                # contribution of local head h (global kt = 4j+h) to batch b's
                # 256 tokens; wo streams in quarter-pass tiles (16KB rows),
                # first quarter ahead of the collective-gated rh gather
                wot0 = wop.tile([128, 8, NCORES, 128], BF16, name="wo")
                nc.sync.dma_start(wot0[:], wo[h, :, 0:8, :, :])
                rh_t = rhp.tile([128, NCORES, SHB], BF16, name="rh")
                nc.sync.dma_start(rh_t[:],
                                  a2a_out[b][h].ap().transpose([1, 0, 2]))
                oa = oacc0 if b == 0 else oacc1
                col0 = b * SHB
                for qtr in range(4):
                    if qtr == 0:
                        wot = wot0
                    else:
                        wot = wop.tile([128, 8, NCORES, 128], BF16, name="wo")
                        nc.sync.dma_start(
                            wot[:], wo[h, :, qtr * 8:(qtr + 1) * 8, :, :])
                    for oti in range(8):
                        ot = qtr * 8 + oti
                        po = ps_o.tile([128, SHB], F32, name="ps_o")
                        for j in range(NCORES):
                            nc.tensor.matmul(po[:], wot[:, oti, j, :],
                                             rh_t[:, j, :],
                                             start=(j == 0),
                                             stop=(j == NCORES - 1))
                        if h == 0:
                            nc.vector.tensor_copy(oa[:, ot, :], po[:])
                        elif h < NH - 1:
                            nc.vector.tensor_add(oa[:, ot, :], po[:],
                                                 oa[:, ot, :])
                        else:
                            o_sb = osb.tile([128, SHB], F32, name="osb")
                            nc.vector.tensor_add(o_sb[:], po[:], oa[:, ot, :])
                            nc.gpsimd.dma_start(
                                out[ot * 128:(ot + 1) * 128, col0:col0 + SHB],
                                o_sb[:])

            attn_head(0, 0, vb0)
            attn_head(0, 1, vb0)
            # b=1's V reloads from DRAM well before b1's attention needs it
            vb1 = vbp.tile([128, LT, OC], BF16, name="vb1")
            nc.sync.dma_start(vb1[:, :, :], vsp1[:, :, :])
            attn_head(0, 2, vb0)
            oproj_pass(0, 0)
            attn_head(0, 3, vb0)
            oproj_pass(0, 1)
            attn_head(1, 0, vb1)
            oproj_pass(0, 2)
            attn_head(1, 1, vb1)
            oproj_pass(0, 3)
            attn_head(1, 2, vb1)
            oproj_pass(1, 0)
            attn_head(1, 3, vb1)
            oproj_pass(1, 1)
            oproj_pass(1, 2)
            oproj_pass(1, 3)

    nc.compile()
    return nc


def _qk_row_perm():
    # local row order: [h0re|h1re],[h0im|h1im],[h2re|h3re],[h2im|h3im]
    rows = []
    for pr in range(NH // 2):
        ha, hb = 2 * pr, 2 * pr + 1
        rows += [ha * HD + 2 * i for i in range(HD // 2)]
        rows += [hb * HD + 2 * i for i in range(HD // 2)]
        rows += [ha * HD + 2 * i + 1 for i in range(HD // 2)]
        rows += [hb * HD + 2 * i + 1 for i in range(HD // 2)]
    return np.array(rows)


def _prep_inputs(x, freqs_cos, freqs_sin, Wq, Wk, Wv, Wo):
    x = np.asarray(x, np.float32).reshape(T, D)
    Wq, Wk, Wv, Wo = (np.asarray(w, np.float32) for w in (Wq, Wk, Wv, Wo))
    fc = np.asarray(freqs_cos, np.float32)
    fs = np.asarray(freqs_sin, np.float32)

    # shared tensors (partition-major pretiling)
    xHh = np.ascontiguousarray(
        x.reshape(NG, TG, KT, 128).transpose(0, 3, 2, 1)).astype(NPBF16)
    woh = np.ascontiguousarray(
        Wo.reshape(D // 128, 128, NCORES, NH, 128).transpose(3, 4, 0, 2, 1)
    ).astype(NPBF16)
    csh = np.ascontiguousarray(np.concatenate([fc.T, fc.T], 0)).astype(np.float16)
    snh = np.ascontiguousarray(np.concatenate([fs.T, fs.T], 0)).astype(np.float16)
    ones = np.ones([128, 128], NPBF16)

    perm = _qk_row_perm()
    in_maps = []
    for i in range(NCORES):
        rows = slice(OC * i, OC * (i + 1))
        wqi = Wq[rows][perm]                                             # [512, D]
        wki = Wk[rows][perm]
        wqh = np.ascontiguousarray(
            wqi.reshape(NH * 128, KT, 128).transpose(2, 1, 0)).astype(NPBF16)
        wkh = np.ascontiguousarray(
            wki.reshape(NH * 128, KT, 128).transpose(2, 1, 0)).astype(NPBF16)
        wvh = np.ascontiguousarray(
            Wv[rows].reshape(OC, KT, 128).transpose(2, 1, 0)).astype(NPBF16)
        in_maps.append({
            "xH": xHh, "wq": wqh, "wk": wkh, "wv": wvh, "wo": woh,
            "cs": csh, "sn": snh, "ones": ones,
        })
    return in_maps


_NC_CACHE = None


def _get_nc():
    global _NC_CACHE
    if _NC_CACHE is None:
        _NC_CACHE = build_nc()
    return _NC_CACHE


def _run(in_maps, trace=False):
    nc = _get_nc()
    res = bass_utils.run_bass_kernel_spmd(
        nc, in_maps, core_ids=list(range(NCORES)), trace=trace)
    return res


def _assemble(results):
    out = np.empty((B, L, D), np.float32)
    for i in range(NCORES):
        o = results[i]["out"]                       # [D, SH] f32
        for b in range(B):
            out[b, SHB * i:SHB * (i + 1), :] = o[:, b * SHB:(b + 1) * SHB].T
    return out


def kernel(x, freqs_cos, freqs_sin, Wq, Wk, Wv, Wo):
    in_maps = _prep_inputs(x, freqs_cos, freqs_sin, Wq, Wk, Wv, Wo)
    res = _run(in_maps, trace=False)
    return _assemble(res.results)
